# revision 85
# baseline (speedup 1.0000x reference)
"""AdaLoRA MLP distributed Trainium2 kernel (8 NeuronCores).

Strategy:
  - Hypernetwork: LN(ada) on every core; W1 sharded by columns (each core
    computes a 128-col slice of hT, AllGather); W2 sharded by columns with a
    factor/r-major permutation (each core computes a (16, 4096) slice of w);
    AllToAll redistributes so each core holds the full LoRA factors for its
    own 2 batches at SPMD-uniform offsets.
  - Main compute: data-parallel over batch (2 per core). x is PE-transposed
    (d onto partitions) for the rank-8 contractions; stage chain
    t1 = x@a1 -> zT = (t1@bb1^T)^T -> gelu -> t2 -> y2 + x.
  - dtypes: W2/h, factors and all stage matmul operands bf16 (f32 PSUM
    accumulation everywhere); x transposes f32.
"""

import sys
import numpy as np

sys.path.insert(0, "/opt/trn_rl_repo")

import ml_dtypes

B, T, D = 16, 1024, 1024
ADA, INTER, RANK = 1024, 1024, 8
NCORES = 8
KA = 1152          # augmented contraction dim (9 * 128): [mat; bias; zeros]
KT = KA // 128     # 9 k-tiles
BPC = B // NCORES  # 2 batches per core
EPS = 1e-5

LAST_EXEC_NS = None
LAST_RESULTS = None


def _build_graph():
    from concourse import bacc, mybir
    from concourse.tile import TileContext
    from concourse.tile import add_dep_helper

    f32 = mybir.dt.float32
    bf16 = mybir.dt.bfloat16

    nc = bacc.Bacc(None, target_bir_lowering=False, debug=False)

    x_ext = nc.declare_dram_parameter("x_sh", [BPC, T, D], bf16, isOutput=False)
    ada_ext = nc.declare_dram_parameter("ada", [B, ADA], f32, isOutput=False)
    w1_ext = nc.declare_dram_parameter("w1s", [KA, INTER], bf16, isOutput=False)
    w2_ext = nc.declare_dram_parameter("w2s", [KA, 4096], mybir.dt.float8e4, isOutput=False)
    id_ext = nc.declare_dram_parameter("ident", [128, 128], f32, isOutput=False)
    out_ext = nc.declare_dram_parameter("out", [BPC, T, D], bf16, isOutput=True)

    # internal DRAM for collectives (collectives cannot touch I/O tensors)
    w_bounce = nc.dram_tensor("w_bounce", [B, 4096], bf16)
    wa2a = nc.dram_tensor("wa2a", [B, 4096], bf16)

    RG = [list(range(NCORES))]
    Gelu = mybir.ActivationFunctionType.Gelu
    Sqrt = mybir.ActivationFunctionType.Sqrt

    with TileContext(nc) as tc:
        with (
            tc.tile_pool(name="const", bufs=1) as cpool,
            tc.tile_pool(name="xp", bufs=8) as xpool,
            tc.tile_pool(name="xtp", bufs=16) as xtpool,
            tc.tile_pool(name="gtp", bufs=12) as gtpool,
            tc.tile_pool(name="w2p", bufs=12) as w2pool,
            tc.tile_pool(name="fctp", bufs=1) as fpool,
            tc.tile_pool(name="stp", bufs=3) as spool,
            tc.tile_pool(name="outp", bufs=4) as opool,
        ):
            # ---------------- constants / small loads ----------------
            ident = cpool.tile([128, 128], f32)
            nc.sync.dma_start(out=ident[:, :], in_=id_ext[:, :])
            identb = cpool.tile([128, 128], bf16)
            nc.vector.tensor_copy(identb[:, :], ident[:, :])
            ada_sb = cpool.tile([B, ADA], f32)
            nc.sync.dma_start(out=ada_sb[:, :], in_=ada_ext[:, :])
            w1_sb = cpool.tile([128, KT, INTER], bf16)
            for w1c in range(3):
                nc.sync.dma_start(
                    out=w1_sb[:, w1c * 3 : (w1c + 1) * 3, :],
                    in_=w1_ext[
                        w1c * 384 : (w1c + 1) * 384, :
                    ].rearrange("(kt p) i -> p kt i", p=128),
                )

            # zero / eps bias tiles (activation() requires AP biases)
            zb = cpool.tile([128, 1], f32)
            nc.vector.memset(zb[:, :], 0.0)
            epsb = cpool.tile([B, 1], f32)
            nc.vector.memset(epsb[:, :], EPS)
            # dummy activation: preload the gelu table before it is needed
            scr1 = cpool.tile([1, 1], f32)
            nc.scalar.activation(
                scr1[:, :], epsb[0:1, 0:1], Gelu, bias=epsb[0:1, 0:1]
            )

            # ---------------- LayerNorm (all 16 rows) ----------------
            # var = E[a^2] - mu^2; normalize fused as a*rstd - mu*rstd
            sq = cpool.tile([B, ADA], f32)
            sum2_c = cpool.tile([B, 1], f32)
            nc.scalar.activation(
                sq[:, :], ada_sb[:, :],
                mybir.ActivationFunctionType.Square,
                bias=zb[0:B, 0:1], accum_out=sum2_c[:, 0:1],
            )
            sum_c = cpool.tile([B, 1], f32)
            nc.vector.tensor_reduce(
                sum_c[:, :], ada_sb[:, :], mybir.AxisListType.X, mybir.AluOpType.add
            )
            ss = cpool.tile([B, 2], f32)
            nc.vector.tensor_scalar_mul(ss[:, 0:1], sum_c[:, :], 1.0 / ADA)
            nc.vector.tensor_scalar_mul(ss[:, 1:2], sum2_c[:, :], 1.0 / ADA)
            mu2_c = cpool.tile([B, 1], f32)
            nc.vector.tensor_mul(mu2_c[:, :], ss[:, 0:1], ss[:, 0:1])
            var_c = cpool.tile([B, 1], f32)
            nc.vector.tensor_sub(var_c[:, :], ss[:, 1:2], mu2_c[:, :])
            # rstd = rsqrt(var + eps) via Newton iteration (var ~= 1 after
            # LN-scale inputs; 3 steps from y0=1 converge for var in (0.1, 2.5))
            nc.vector.tensor_scalar_add(var_c[:, :], var_c[:, :], EPS)
            yt = cpool.tile([B, 1], f32)
            ht_ = cpool.tile([B, 1], f32)
            nc.vector.tensor_scalar(
                yt[:, :], var_c[:, :], -0.5, 1.5,
                mybir.AluOpType.mult, mybir.AluOpType.add,
            )
            for _it in range(2):
                # y <- y * (1.5 - 0.5 * v * y^2)
                nc.vector.tensor_mul(ht_[:, :], yt[:, :], yt[:, :])
                nc.vector.tensor_mul(ht_[:, :], ht_[:, :], var_c[:, :])
                nc.vector.tensor_scalar(
                    ht_[:, :], ht_[:, :], -0.5, 1.5,
                    mybir.AluOpType.mult, mybir.AluOpType.add,
                )
                nc.vector.tensor_mul(yt[:, :], yt[:, :], ht_[:, :])
            murstd_c = cpool.tile([B, 1], f32)
            nc.vector.tensor_mul(murstd_c[:, :], ss[:, 0:1], yt[:, :])
            alnr = cpool.tile([B, ADA], f32)
            nc.vector.tensor_scalar(
                alnr[:, :], ada_sb[:, :], yt[:, 0:1], murstd_c[:, 0:1],
                mybir.AluOpType.mult, mybir.AluOpType.subtract,
            )

            # transpose alnr (16, 1024) -> alnT (128, 9, 16) bf16 + aug
            alnT = cpool.tile([128, KT, B], bf16)
            h_nat = cpool.tile([B, INTER], bf16)
            h_sb = cpool.tile([128, KT, B], bf16)
            with (
                tc.tile_pool(name="psmall", bufs=3, space="PSUM") as psmall,
                tc.tile_pool(name="ph1", bufs=1, space="PSUM") as ph1,
                tc.tile_pool(name="ph2", bufs=3, space="PSUM") as ph2,
            ):
                for k in range(8):
                    tp = psmall.tile([128, B], f32, tag="palnt")
                    nc.tensor.transpose(
                        tp[:, :],
                        alnr[:, k * 128 : (k + 1) * 128],
                        ident[0:B, 0:B],
                    )
                    nc.vector.tensor_copy(alnT[:, k, :], tp[:, :])
                nc.vector.memset(alnT[:, 8, :], 0.0)
                nc.vector.memset(alnT[0:1, 8, :], 1.0)

                # ---- W1 phase (replicated): h = gelu(alnT^T @ W1aug) ----
                h_ps = ph1.tile([B, INTER], f32, tag="pht")
                for kt in range(KT):
                    for c in range(2):
                        nc.tensor.matmul(
                            h_ps[:, c * 512 : (c + 1) * 512],
                            alnT[:, kt, :],
                            w1_sb[:, kt, c * 512 : (c + 1) * 512],
                            start=(kt == 0),
                            stop=(kt == KT - 1),
                        )
                nc.scalar.activation(
                    h_nat[:, :], h_ps[:, :], Gelu, bias=zb[0:B, 0:1]
                )
                # keep PE warm across the gelu gap (pstate continuity)
                warm_sc = cpool.tile([128, 48], bf16)
                for wv in range(3):
                    tpw = ph2.tile([128, B], bf16, tag="phtT")
                    nc.tensor.transpose(
                        tpw[:, :], identb[0:B, :], identb[0:B, 0:B]
                    )
                    nc.vector.tensor_copy(
                        warm_sc[:, wv * 16 : (wv + 1) * 16], tpw[:, :]
                    )

                # ---- transpose h -> hT (128, 9, 16) bf16 + aug tile ----
                for k in range(8):
                    tph = ph2.tile([128, B], bf16, tag="phtT")
                    nc.tensor.transpose(
                        tph[:, :],
                        h_nat[:, k * 128 : (k + 1) * 128],
                        identb[0:B, 0:B],
                    )
                    nc.vector.tensor_copy(h_sb[:, k, :], tph[:, :])
            nc.vector.memset(h_sb[:, 8, :], 0.0)
            nc.vector.memset(h_sb[0:1, 8, :], 1.0)

            # -------- W2 phase, x loads/transposes, A2A, factors --------
            x_sb = {}
            xt_sb = {}
            fct = {}
            a_d = {}
            with (
                tc.tile_pool(name="pxt", bufs=2, space="PSUM") as pxt,
                tc.tile_pool(name="ppat", bufs=1, space="PSUM") as ppat,
            ):
                last_w2_dma = [None]

                def load_x_batch(q, gate=True):
                    for tp2 in range(4):
                        xt_ = xpool.tile([128, 2, D], bf16, tag="x")
                        xdma = nc.sync.dma_start(
                            out=xt_[:, :, :],
                            in_=x_ext[
                                q, tp2 * 256 : (tp2 + 1) * 256, :
                            ].rearrange("(j p) d -> p j d", p=128),
                        )
                        if gate and last_w2_dma[0] is not None:
                            add_dep_helper(
                                xdma.ins,
                                last_w2_dma[0],
                                sync=True,
                                reason="x loads yield DMA bw to W2",
                            )  # gating disabled at call sites below
                        x_sb[(q, 2 * tp2)] = xt_[:, 0, :]
                        x_sb[(q, 2 * tp2 + 1)] = xt_[:, 1, :]

                def transpose_batch(q):
                    for dc in range(8):
                        xt_t = xtpool.tile([128, D], bf16, tag="xt")
                        for half in range(2):
                            tp = pxt.tile([128, 512], bf16, tag="xt")
                            for i in range(4):
                                tt = half * 4 + i
                                nc.tensor.transpose(
                                    tp[:, i * 128 : (i + 1) * 128],
                                    x_sb[(q, tt)][:, dc * 128 : (dc + 1) * 128],
                                    identb[:, :],
                                )
                            nc.vector.tensor_copy(
                                xt_t[:, half * 512 : (half + 1) * 512], tp[:, :]
                            )
                        xt_sb[(q, dc)] = xt_t

                def load_factors(q):
                    ft = fpool.tile([RANK, 4, 1024], bf16, tag=f"fct{q}")
                    for rh in range(2):
                        nc.scalar.dma_start(
                            out=ft[rh * 4 : rh * 4 + 4, :, :],
                            in_=wa2a[q + 2 * rh :: 4, :].rearrange(
                                "f (rl d) -> rl f d", rl=4
                            ),
                        )
                    fct[q] = ft

                def make_ad(q):
                    for fi in (0, 2):
                        ad = spool.tile([128, 64], bf16, tag=f"ad{fi}_{q}")
                        tp = ppat.tile([128, 64], bf16, tag="pat")
                        for dc in range(8):
                            nc.tensor.transpose(
                                tp[:, dc * 8 : (dc + 1) * 8],
                                fct[q][:, fi, dc * 128 : (dc + 1) * 128],
                                identb[0:RANK, 0:RANK],
                            )
                        nc.vector.tensor_copy(ad[:, :], tp[:, :])
                        a_d[(q, fi)] = ad

                with tc.tile_pool(name="pw2", bufs=2, space="PSUM") as pw2:
                    for half in range(2):
                        w_ps = []
                        for _wj in range(2):
                            w_ps_j = pw2.tile([B, 1024], f32, tag="w")
                            w_ps.append(w_ps_j)
                        for kt in range(KT):
                            wt = w2pool.tile([128, 2048], mybir.dt.float8e4, tag="w2")
                            wdma = nc.sync.dma_start(
                                out=wt[:, :],
                                in_=w2_ext[
                                    kt * 128 : (kt + 1) * 128,
                                    half * 2048 : (half + 1) * 2048,
                                ],
                            )
                            last_w2_dma[0] = wdma.ins
                            for j in range(2):
                                for c in range(2):
                                    nc.tensor.matmul(
                                        w_ps[j][:, c * 512 : (c + 1) * 512],
                                        h_sb[:, kt, :],
                                        wt[
                                            :,
                                            j * 1024
                                            + c * 512 : j * 1024
                                            + (c + 1) * 512,
                                        ],
                                        start=(kt == 0),
                                        stop=(kt == KT - 1),
                                    )
                        for j in range(2):
                            w_sbp = spool.tile([B, 1024], bf16, tag="wp")
                            nc.vector.tensor_scalar_mul(
                                w_sbp[:, 0:512], w_ps[j][:, 0:512], 1.0 / 64.0
                            )
                            nc.scalar.activation(
                                w_sbp[:, 512:1024],
                                w_ps[j][:, 512:1024],
                                mybir.ActivationFunctionType.Copy,
                                scale=1.0 / 64.0,
                            )
                            wbdma = nc.scalar.dma_start(
                                out=w_bounce[
                                    :,
                                    (half * 2 + j) * 1024 : (half * 2 + j + 1)
                                    * 1024,
                                ],
                                in_=w_sbp[:, :],
                            )
                            last_w2_dma[0] = wbdma.ins

                load_x_batch(0)
                load_x_batch(1)
                transpose_batch(0)
                transpose_batch(1)

                nc.gpsimd.collective_compute(
                    "AllToAll",
                    mybir.AluOpType.bypass,
                    replica_groups=RG,
                    ins=[w_bounce.ap().opt()],
                    outs=[wa2a.ap().opt()],
                )

                load_factors(0)
                make_ad(0)
                load_factors(1)
                make_ad(1)

            # -------- main stages (software-pipelined across batches) ----
            with (
                tc.tile_pool(name="pt", bufs=2, space="PSUM") as pt,
                tc.tile_pool(name="pzy", bufs=3, space="PSUM") as pzy,
            ):
                t1_sb, t2_sb, gt = {}, {}, {}

                def do_st1(q):
                    t1q = spool.tile([RANK, T], bf16, tag="t1")
                    for c in range(2):
                        t1_ps = pt.tile([RANK, 512], f32, tag="t")
                        for dc in range(8):
                            nc.tensor.matmul(
                                t1_ps[:, :],
                                a_d[(q, 0)][:, dc * 8 : (dc + 1) * 8],
                                xt_sb[(q, dc)][:, c * 512 : (c + 1) * 512],
                                start=(dc == 0),
                                stop=(dc == 7),
                            )
                        nc.vector.tensor_copy(
                            t1q[:, c * 512 : (c + 1) * 512], t1_ps[:, :]
                        )
                    t1_sb[q] = t1q

                def do_st2(q):
                    for dc in range(8):
                        z_ps = pzy.tile([128, T], f32, tag="zy")
                        for c in range(2):
                            nc.tensor.matmul(
                                z_ps[:, c * 512 : (c + 1) * 512],
                                fct[q][:, 1, dc * 128 : (dc + 1) * 128],
                                t1_sb[q][:, c * 512 : (c + 1) * 512],
                                start=True,
                                stop=True,
                            )
                        g_t = gtpool.tile([128, T], bf16, tag="gt")
                        nc.scalar.activation(
                            g_t[:, :], z_ps[:, :], Gelu, bias=zb[:, 0:1]
                        )
                        gt[(q, dc)] = g_t

                def do_st3(q):
                    t2q = spool.tile([RANK, T], bf16, tag="t2")
                    for c in range(2):
                        t2_ps = pt.tile([RANK, 512], f32, tag="t")
                        for dc in range(8):
                            nc.tensor.matmul(
                                t2_ps[:, :],
                                a_d[(q, 2)][:, dc * 8 : (dc + 1) * 8],
                                gt[(q, dc)][:, c * 512 : (c + 1) * 512],
                                start=(dc == 0),
                                stop=(dc == 7),
                            )
                        nc.vector.tensor_copy(
                            t2q[:, c * 512 : (c + 1) * 512], t2_ps[:, :]
                        )
                    t2_sb[q] = t2q

                def do_st4(q):
                    o2 = None
                    for tt in range(8):
                        y_ps = pzy.tile([128, D], f32, tag="zy")
                        inject = tt % 2 == 0
                        for c in range(2):
                            if inject:
                                # x into PSUM, accumulate y2, drain on ACT
                                nc.tensor.matmul(
                                    y_ps[:, c * 512 : (c + 1) * 512],
                                    identb[:, :],
                                    x_sb[(q, tt)][:, c * 512 : (c + 1) * 512],
                                    start=True,
                                    stop=False,
                                )
                            nc.tensor.matmul(
                                y_ps[:, c * 512 : (c + 1) * 512],
                                t2_sb[q][:, tt * 128 : (tt + 1) * 128],
                                fct[q][:, 3, c * 512 : (c + 1) * 512],
                                start=not inject,
                                stop=True,
                            )
                        if tt % 2 == 0:
                            o2 = opool.tile([128, 2, D], bf16, tag="o")
                        o_slice = o2[:, tt % 2, :]
                        if inject:
                            nc.scalar.activation(
                                o_slice,
                                y_ps[:, :],
                                mybir.ActivationFunctionType.Copy,
                            )
                        else:
                            nc.vector.tensor_add(
                                o_slice, y_ps[:, :], x_sb[(q, tt)][:, :]
                            )
                        if tt % 2 == 1:
                            nc.sync.dma_start(
                                out=out_ext[
                                    q, (tt - 1) * 128 : (tt + 1) * 128, :
                                ].rearrange("(j p) d -> p j d", p=128),
                                in_=o2[:, :, :],
                            )

                do_st1(0)
                do_st2(0)
                do_st1(1)
                do_st2(1)
                do_st3(0)
                do_st4(0)
                do_st3(1)
                do_st4(1)

    nc.compile()
    return nc


def _prep_inputs(x, ada_emb, ln_g, ln_b, W1, b1, W2, b2):
    f32 = np.float32
    x = np.ascontiguousarray(np.asarray(x, dtype=f32))
    ada = np.ascontiguousarray(np.asarray(ada_emb, dtype=f32))
    ln_g = np.asarray(ln_g, dtype=f32)
    ln_b = np.asarray(ln_b, dtype=f32)
    W1 = np.asarray(W1, dtype=f32)
    b1 = np.asarray(b1, dtype=f32)
    W2 = np.asarray(W2, dtype=f32)
    b2 = np.asarray(b2, dtype=f32)

    # fold LN affine into W1
    W1f = W1 * ln_g[:, None]
    b1f = b1 + ln_b @ W1
    W1aug = np.zeros((KA, INTER), dtype=f32)
    W1aug[:ADA] = W1f
    W1aug[ADA] = b1f

    W2aug = np.zeros((KA, 4 * D * RANK), dtype=f32)
    W2aug[:INTER] = W2
    W2aug[INTER] = b2

    ident = np.ascontiguousarray(np.eye(128, dtype=f32))
    w1aug_bf = np.ascontiguousarray(W1aug).astype(ml_dtypes.bfloat16)

    j = np.arange(4096)
    in_maps = []
    for c in range(NCORES):
        f, rh = c // 2, c % 2
        cols = f * 8192 + (j % 1024) * RANK + 4 * rh + j // 1024
        w2s = np.ascontiguousarray(W2aug[:, cols] * 64.0).astype(
            ml_dtypes.float8_e4m3
        )
        in_maps.append(
            {
                "x_sh": np.ascontiguousarray(x[BPC * c : BPC * (c + 1)]).astype(ml_dtypes.bfloat16),
                "ada": ada,
                "w1s": w1aug_bf,
                "w2s": w2s,
                "ident": ident,
            }
        )
    return in_maps


def kernel(x, ada_emb, ln_g, ln_b, W1, b1, W2, b2):
    global LAST_EXEC_NS, LAST_RESULTS
    from concourse.bass_utils import run_bass_kernel_spmd

    nc = _build_graph()
    in_maps = _prep_inputs(x, ada_emb, ln_g, ln_b, W1, b1, W2, b2)

    trace = bool(int(__import__("os").environ.get("KTRACE", "0")))
    res = run_bass_kernel_spmd(
        nc, in_maps, core_ids=list(range(NCORES)), trace=trace
    )
    LAST_EXEC_NS = res.exec_time_ns
    LAST_RESULTS = res

    out = np.empty((B, T, D), dtype=np.float32)
    for c in range(NCORES):
        out[BPC * c : BPC * (c + 1)] = res.results[c]["out"].astype(np.float32)
    return out


# revision 86
# speedup vs baseline: 1.0026x; 1.0026x over previous
"""AdaLoRA MLP distributed Trainium2 kernel (8 NeuronCores).

Strategy:
  - Hypernetwork: LN(ada) on every core; W1 sharded by columns (each core
    computes a 128-col slice of hT, AllGather); W2 sharded by columns with a
    factor/r-major permutation (each core computes a (16, 4096) slice of w);
    AllToAll redistributes so each core holds the full LoRA factors for its
    own 2 batches at SPMD-uniform offsets.
  - Main compute: data-parallel over batch (2 per core). x is PE-transposed
    (d onto partitions) for the rank-8 contractions; stage chain
    t1 = x@a1 -> zT = (t1@bb1^T)^T -> gelu -> t2 -> y2 + x.
  - dtypes: W2/h, factors and all stage matmul operands bf16 (f32 PSUM
    accumulation everywhere); x transposes f32.
"""

import sys
import numpy as np

sys.path.insert(0, "/opt/trn_rl_repo")

import ml_dtypes

B, T, D = 16, 1024, 1024
ADA, INTER, RANK = 1024, 1024, 8
NCORES = 8
KA = 1152          # augmented contraction dim (9 * 128): [mat; bias; zeros]
KT = KA // 128     # 9 k-tiles
BPC = B // NCORES  # 2 batches per core
EPS = 1e-5

LAST_EXEC_NS = None
LAST_RESULTS = None


def _build_graph():
    from concourse import bacc, mybir
    from concourse.tile import TileContext
    from concourse.tile import add_dep_helper

    f32 = mybir.dt.float32
    bf16 = mybir.dt.bfloat16

    nc = bacc.Bacc(None, target_bir_lowering=False, debug=False)

    x_ext = nc.declare_dram_parameter("x_sh", [BPC, T, D], bf16, isOutput=False)
    ada_ext = nc.declare_dram_parameter("ada", [B, ADA], f32, isOutput=False)
    w1_ext = nc.declare_dram_parameter("w1s", [KA, INTER], bf16, isOutput=False)
    w2_ext = nc.declare_dram_parameter("w2s", [KA, 4096], mybir.dt.float8e4, isOutput=False)
    id_ext = nc.declare_dram_parameter("ident", [128, 128], f32, isOutput=False)
    out_ext = nc.declare_dram_parameter("out", [BPC, T, D], bf16, isOutput=True)

    # internal DRAM for collectives (collectives cannot touch I/O tensors)
    w_bounce = nc.dram_tensor("w_bounce", [B, 4096], bf16)
    wa2a = nc.dram_tensor("wa2a", [B, 4096], bf16)

    RG = [list(range(NCORES))]
    Gelu = mybir.ActivationFunctionType.Gelu
    Sqrt = mybir.ActivationFunctionType.Sqrt

    with TileContext(nc) as tc:
        with (
            tc.tile_pool(name="const", bufs=1) as cpool,
            tc.tile_pool(name="xp", bufs=8) as xpool,
            tc.tile_pool(name="xtp", bufs=16) as xtpool,
            tc.tile_pool(name="gtp", bufs=12) as gtpool,
            tc.tile_pool(name="w2p", bufs=12) as w2pool,
            tc.tile_pool(name="fctp", bufs=1) as fpool,
            tc.tile_pool(name="stp", bufs=3) as spool,
            tc.tile_pool(name="outp", bufs=4) as opool,
        ):
            # ---------------- constants / small loads ----------------
            ident = cpool.tile([128, 128], f32)
            nc.sync.dma_start(out=ident[:, :], in_=id_ext[:, :])
            identb = cpool.tile([128, 128], bf16)
            nc.vector.tensor_copy(identb[:, :], ident[:, :])
            ada_sb = cpool.tile([B, ADA], f32)
            nc.sync.dma_start(out=ada_sb[:, :], in_=ada_ext[:, :])
            w1_sb = cpool.tile([128, KT, INTER], bf16)
            for w1c in range(3):
                nc.sync.dma_start(
                    out=w1_sb[:, w1c * 3 : (w1c + 1) * 3, :],
                    in_=w1_ext[
                        w1c * 384 : (w1c + 1) * 384, :
                    ].rearrange("(kt p) i -> p kt i", p=128),
                )

            # zero / eps bias tiles (activation() requires AP biases)
            zb = cpool.tile([128, 1], f32)
            nc.vector.memset(zb[:, :], 0.0)
            epsb = cpool.tile([B, 1], f32)
            nc.vector.memset(epsb[:, :], EPS)
            # dummy activation: preload the gelu table before it is needed
            scr1 = cpool.tile([1, 1], f32)
            nc.scalar.activation(
                scr1[:, :], epsb[0:1, 0:1], Gelu, bias=epsb[0:1, 0:1]
            )

            # ---------------- LayerNorm (all 16 rows) ----------------
            # var = E[a^2] - mu^2; normalize fused as a*rstd - mu*rstd
            sq = cpool.tile([B, ADA], f32)
            sum2_c = cpool.tile([B, 1], f32)
            nc.scalar.activation(
                sq[:, :], ada_sb[:, :],
                mybir.ActivationFunctionType.Square,
                bias=zb[0:B, 0:1], accum_out=sum2_c[:, 0:1],
            )
            sum_c = cpool.tile([B, 1], f32)
            nc.vector.tensor_reduce(
                sum_c[:, :], ada_sb[:, :], mybir.AxisListType.X, mybir.AluOpType.add
            )
            ss = cpool.tile([B, 2], f32)
            nc.vector.tensor_scalar_mul(ss[:, 0:1], sum_c[:, :], 1.0 / ADA)
            nc.vector.tensor_scalar_mul(ss[:, 1:2], sum2_c[:, :], 1.0 / ADA)
            mu2_c = cpool.tile([B, 1], f32)
            nc.vector.tensor_mul(mu2_c[:, :], ss[:, 0:1], ss[:, 0:1])
            var_c = cpool.tile([B, 1], f32)
            nc.vector.tensor_sub(var_c[:, :], ss[:, 1:2], mu2_c[:, :])
            # rstd = rsqrt(var + eps) via Newton iteration (var ~= 1 after
            # LN-scale inputs; 3 steps from y0=1 converge for var in (0.1, 2.5))
            nc.vector.tensor_scalar_add(var_c[:, :], var_c[:, :], EPS)
            yt = cpool.tile([B, 1], f32)
            ht_ = cpool.tile([B, 1], f32)
            nc.vector.tensor_scalar(
                yt[:, :], var_c[:, :], -0.5, 1.5,
                mybir.AluOpType.mult, mybir.AluOpType.add,
            )
            for _it in range(2):
                # y <- y * (1.5 - 0.5 * v * y^2)
                nc.vector.tensor_mul(ht_[:, :], yt[:, :], yt[:, :])
                nc.vector.tensor_mul(ht_[:, :], ht_[:, :], var_c[:, :])
                nc.vector.tensor_scalar(
                    ht_[:, :], ht_[:, :], -0.5, 1.5,
                    mybir.AluOpType.mult, mybir.AluOpType.add,
                )
                nc.vector.tensor_mul(yt[:, :], yt[:, :], ht_[:, :])
            murstd_c = cpool.tile([B, 1], f32)
            nc.vector.tensor_mul(murstd_c[:, :], ss[:, 0:1], yt[:, :])
            alnr = cpool.tile([B, ADA], f32)
            nc.vector.tensor_scalar(
                alnr[:, :], ada_sb[:, :], yt[:, 0:1], murstd_c[:, 0:1],
                mybir.AluOpType.mult, mybir.AluOpType.subtract,
            )

            # transpose alnr (16, 1024) -> alnT (128, 9, 16) bf16 + aug
            alnT = cpool.tile([128, KT, B], bf16)
            h_nat = cpool.tile([B, INTER], bf16)
            h_sb = cpool.tile([128, KT, B], bf16)
            with (
                tc.tile_pool(name="psmall", bufs=2, space="PSUM") as psmall,
                tc.tile_pool(name="ph1", bufs=1, space="PSUM") as ph1,
                tc.tile_pool(name="ph2", bufs=4, space="PSUM") as ph2,
            ):
                for k in range(8):
                    tp = psmall.tile([128, B], f32, tag="palnt")
                    nc.tensor.transpose(
                        tp[:, :],
                        alnr[:, k * 128 : (k + 1) * 128],
                        ident[0:B, 0:B],
                    )
                    nc.vector.tensor_copy(alnT[:, k, :], tp[:, :])
                nc.vector.memset(alnT[:, 8, :], 0.0)
                nc.vector.memset(alnT[0:1, 8, :], 1.0)

                # ---- W1 phase (replicated): h = gelu(alnT^T @ W1aug) ----
                h_ps = ph1.tile([B, INTER], f32, tag="pht")
                for kt in range(KT):
                    for c in range(2):
                        nc.tensor.matmul(
                            h_ps[:, c * 512 : (c + 1) * 512],
                            alnT[:, kt, :],
                            w1_sb[:, kt, c * 512 : (c + 1) * 512],
                            start=(kt == 0),
                            stop=(kt == KT - 1),
                        )
                nc.scalar.activation(
                    h_nat[:, :], h_ps[:, :], Gelu, bias=zb[0:B, 0:1]
                )
                # keep PE warm across the gelu gap (pstate continuity)
                warm_sc = cpool.tile([128, 48], bf16)
                for wv in range(3):
                    tpw = ph2.tile([128, B], bf16, tag="phtT")
                    nc.tensor.transpose(
                        tpw[:, :], identb[0:B, :], identb[0:B, 0:B]
                    )
                    nc.vector.tensor_copy(
                        warm_sc[:, wv * 16 : (wv + 1) * 16], tpw[:, :]
                    )

                # ---- transpose h -> hT (128, 9, 16) bf16 + aug tile ----
                for k in range(8):
                    tph = ph2.tile([128, B], bf16, tag="phtT")
                    nc.tensor.transpose(
                        tph[:, :],
                        h_nat[:, k * 128 : (k + 1) * 128],
                        identb[0:B, 0:B],
                    )
                    nc.vector.tensor_copy(h_sb[:, k, :], tph[:, :])
            nc.vector.memset(h_sb[:, 8, :], 0.0)
            nc.vector.memset(h_sb[0:1, 8, :], 1.0)

            # -------- W2 phase, x loads/transposes, A2A, factors --------
            x_sb = {}
            xt_sb = {}
            fct = {}
            a_d = {}
            with (
                tc.tile_pool(name="pxt", bufs=2, space="PSUM") as pxt,
                tc.tile_pool(name="ppat", bufs=1, space="PSUM") as ppat,
            ):
                last_w2_dma = [None]

                def load_x_batch(q, gate=True):
                    for tp2 in range(4):
                        xt_ = xpool.tile([128, 2, D], bf16, tag="x")
                        xdma = nc.sync.dma_start(
                            out=xt_[:, :, :],
                            in_=x_ext[
                                q, tp2 * 256 : (tp2 + 1) * 256, :
                            ].rearrange("(j p) d -> p j d", p=128),
                        )
                        if gate and last_w2_dma[0] is not None:
                            add_dep_helper(
                                xdma.ins,
                                last_w2_dma[0],
                                sync=True,
                                reason="x loads yield DMA bw to W2",
                            )  # gating disabled at call sites below
                        x_sb[(q, 2 * tp2)] = xt_[:, 0, :]
                        x_sb[(q, 2 * tp2 + 1)] = xt_[:, 1, :]

                def transpose_batch(q):
                    for dc in range(8):
                        xt_t = xtpool.tile([128, D], bf16, tag="xt")
                        for half in range(2):
                            tp = pxt.tile([128, 512], bf16, tag="xt")
                            for i in range(4):
                                tt = half * 4 + i
                                nc.tensor.transpose(
                                    tp[:, i * 128 : (i + 1) * 128],
                                    x_sb[(q, tt)][:, dc * 128 : (dc + 1) * 128],
                                    identb[:, :],
                                )
                            nc.vector.tensor_copy(
                                xt_t[:, half * 512 : (half + 1) * 512], tp[:, :]
                            )
                        xt_sb[(q, dc)] = xt_t

                def load_factors(q):
                    ft = fpool.tile([RANK, 4, 1024], bf16, tag=f"fct{q}")
                    for rh in range(2):
                        nc.scalar.dma_start(
                            out=ft[rh * 4 : rh * 4 + 4, :, :],
                            in_=wa2a[q + 2 * rh :: 4, :].rearrange(
                                "f (rl d) -> rl f d", rl=4
                            ),
                        )
                    fct[q] = ft

                def make_ad(q):
                    for fi in (0, 2):
                        ad = spool.tile([128, 64], bf16, tag=f"ad{fi}_{q}")
                        tp = ppat.tile([128, 64], bf16, tag="pat")
                        for dc in range(8):
                            nc.tensor.transpose(
                                tp[:, dc * 8 : (dc + 1) * 8],
                                fct[q][:, fi, dc * 128 : (dc + 1) * 128],
                                identb[0:RANK, 0:RANK],
                            )
                        nc.vector.tensor_copy(ad[:, :], tp[:, :])
                        a_d[(q, fi)] = ad

                with tc.tile_pool(name="pw2", bufs=2, space="PSUM") as pw2:
                    for half in range(2):
                        w_ps = []
                        for _wj in range(2):
                            w_ps_j = pw2.tile([B, 1024], f32, tag="w")
                            w_ps.append(w_ps_j)
                        for kt in range(KT):
                            wt = w2pool.tile([128, 2048], mybir.dt.float8e4, tag="w2")
                            wdma = nc.sync.dma_start(
                                out=wt[:, :],
                                in_=w2_ext[
                                    kt * 128 : (kt + 1) * 128,
                                    half * 2048 : (half + 1) * 2048,
                                ],
                            )
                            last_w2_dma[0] = wdma.ins
                            for j in range(2):
                                for c in range(2):
                                    nc.tensor.matmul(
                                        w_ps[j][:, c * 512 : (c + 1) * 512],
                                        h_sb[:, kt, :],
                                        wt[
                                            :,
                                            j * 1024
                                            + c * 512 : j * 1024
                                            + (c + 1) * 512,
                                        ],
                                        start=(kt == 0),
                                        stop=(kt == KT - 1),
                                    )
                        for j in range(2):
                            w_sbp = spool.tile([B, 1024], bf16, tag="wp")
                            nc.vector.tensor_scalar_mul(
                                w_sbp[:, 0:512], w_ps[j][:, 0:512], 1.0 / 64.0
                            )
                            nc.scalar.activation(
                                w_sbp[:, 512:1024],
                                w_ps[j][:, 512:1024],
                                mybir.ActivationFunctionType.Copy,
                                scale=1.0 / 64.0,
                            )
                            wbdma = nc.scalar.dma_start(
                                out=w_bounce[
                                    :,
                                    (half * 2 + j) * 1024 : (half * 2 + j + 1)
                                    * 1024,
                                ],
                                in_=w_sbp[:, :],
                            )
                            last_w2_dma[0] = wbdma.ins

                load_x_batch(0)
                load_x_batch(1)
                transpose_batch(0)
                transpose_batch(1)

                nc.gpsimd.collective_compute(
                    "AllToAll",
                    mybir.AluOpType.bypass,
                    replica_groups=RG,
                    ins=[w_bounce.ap().opt()],
                    outs=[wa2a.ap().opt()],
                )

                load_factors(0)
                make_ad(0)
                load_factors(1)
                make_ad(1)

            # -------- main stages (software-pipelined across batches) ----
            with (
                tc.tile_pool(name="pt", bufs=2, space="PSUM") as pt,
                tc.tile_pool(name="pzy", bufs=3, space="PSUM") as pzy,
            ):
                t1_sb, t2_sb, gt = {}, {}, {}

                def do_st1(q):
                    t1q = spool.tile([RANK, T], bf16, tag="t1")
                    for c in range(2):
                        t1_ps = pt.tile([RANK, 512], f32, tag="t")
                        for dc in range(8):
                            nc.tensor.matmul(
                                t1_ps[:, :],
                                a_d[(q, 0)][:, dc * 8 : (dc + 1) * 8],
                                xt_sb[(q, dc)][:, c * 512 : (c + 1) * 512],
                                start=(dc == 0),
                                stop=(dc == 7),
                            )
                        nc.vector.tensor_copy(
                            t1q[:, c * 512 : (c + 1) * 512], t1_ps[:, :]
                        )
                    t1_sb[q] = t1q

                def do_st2(q):
                    for dc in range(8):
                        z_ps = pzy.tile([128, T], f32, tag="zy")
                        for c in range(2):
                            nc.tensor.matmul(
                                z_ps[:, c * 512 : (c + 1) * 512],
                                fct[q][:, 1, dc * 128 : (dc + 1) * 128],
                                t1_sb[q][:, c * 512 : (c + 1) * 512],
                                start=True,
                                stop=True,
                            )
                        g_t = gtpool.tile([128, T], bf16, tag="gt")
                        nc.scalar.activation(
                            g_t[:, :], z_ps[:, :], Gelu, bias=zb[:, 0:1]
                        )
                        gt[(q, dc)] = g_t

                def do_st3(q):
                    t2q = spool.tile([RANK, T], bf16, tag="t2")
                    for c in range(2):
                        t2_ps = pt.tile([RANK, 512], f32, tag="t")
                        for dc in range(8):
                            nc.tensor.matmul(
                                t2_ps[:, :],
                                a_d[(q, 2)][:, dc * 8 : (dc + 1) * 8],
                                gt[(q, dc)][:, c * 512 : (c + 1) * 512],
                                start=(dc == 0),
                                stop=(dc == 7),
                            )
                        nc.vector.tensor_copy(
                            t2q[:, c * 512 : (c + 1) * 512], t2_ps[:, :]
                        )
                    t2_sb[q] = t2q

                def do_st4(q):
                    o2 = None
                    for tt in range(8):
                        y_ps = pzy.tile([128, D], f32, tag="zy")
                        inject = tt % 2 == 0
                        for c in range(2):
                            if inject:
                                # x into PSUM, accumulate y2, drain on ACT
                                nc.tensor.matmul(
                                    y_ps[:, c * 512 : (c + 1) * 512],
                                    identb[:, :],
                                    x_sb[(q, tt)][:, c * 512 : (c + 1) * 512],
                                    start=True,
                                    stop=False,
                                )
                            nc.tensor.matmul(
                                y_ps[:, c * 512 : (c + 1) * 512],
                                t2_sb[q][:, tt * 128 : (tt + 1) * 128],
                                fct[q][:, 3, c * 512 : (c + 1) * 512],
                                start=not inject,
                                stop=True,
                            )
                        if tt % 2 == 0:
                            o2 = opool.tile([128, 2, D], bf16, tag="o")
                        o_slice = o2[:, tt % 2, :]
                        if inject:
                            nc.scalar.activation(
                                o_slice,
                                y_ps[:, :],
                                mybir.ActivationFunctionType.Copy,
                            )
                        else:
                            nc.vector.tensor_add(
                                o_slice, y_ps[:, :], x_sb[(q, tt)][:, :]
                            )
                        if tt % 2 == 1:
                            nc.sync.dma_start(
                                out=out_ext[
                                    q, (tt - 1) * 128 : (tt + 1) * 128, :
                                ].rearrange("(j p) d -> p j d", p=128),
                                in_=o2[:, :, :],
                            )

                do_st1(0)
                do_st2(0)
                do_st1(1)
                do_st2(1)
                do_st3(0)
                do_st4(0)
                do_st3(1)
                do_st4(1)

    nc.compile()
    return nc


def _prep_inputs(x, ada_emb, ln_g, ln_b, W1, b1, W2, b2):
    f32 = np.float32
    x = np.ascontiguousarray(np.asarray(x, dtype=f32))
    ada = np.ascontiguousarray(np.asarray(ada_emb, dtype=f32))
    ln_g = np.asarray(ln_g, dtype=f32)
    ln_b = np.asarray(ln_b, dtype=f32)
    W1 = np.asarray(W1, dtype=f32)
    b1 = np.asarray(b1, dtype=f32)
    W2 = np.asarray(W2, dtype=f32)
    b2 = np.asarray(b2, dtype=f32)

    # fold LN affine into W1
    W1f = W1 * ln_g[:, None]
    b1f = b1 + ln_b @ W1
    W1aug = np.zeros((KA, INTER), dtype=f32)
    W1aug[:ADA] = W1f
    W1aug[ADA] = b1f

    W2aug = np.zeros((KA, 4 * D * RANK), dtype=f32)
    W2aug[:INTER] = W2
    W2aug[INTER] = b2

    ident = np.ascontiguousarray(np.eye(128, dtype=f32))
    w1aug_bf = np.ascontiguousarray(W1aug).astype(ml_dtypes.bfloat16)

    j = np.arange(4096)
    in_maps = []
    for c in range(NCORES):
        f, rh = c // 2, c % 2
        cols = f * 8192 + (j % 1024) * RANK + 4 * rh + j // 1024
        w2s = np.ascontiguousarray(W2aug[:, cols] * 64.0).astype(
            ml_dtypes.float8_e4m3
        )
        in_maps.append(
            {
                "x_sh": np.ascontiguousarray(x[BPC * c : BPC * (c + 1)]).astype(ml_dtypes.bfloat16),
                "ada": ada,
                "w1s": w1aug_bf,
                "w2s": w2s,
                "ident": ident,
            }
        )
    return in_maps


def kernel(x, ada_emb, ln_g, ln_b, W1, b1, W2, b2):
    global LAST_EXEC_NS, LAST_RESULTS
    from concourse.bass_utils import run_bass_kernel_spmd

    nc = _build_graph()
    in_maps = _prep_inputs(x, ada_emb, ln_g, ln_b, W1, b1, W2, b2)

    trace = bool(int(__import__("os").environ.get("KTRACE", "0")))
    res = run_bass_kernel_spmd(
        nc, in_maps, core_ids=list(range(NCORES)), trace=trace
    )
    LAST_EXEC_NS = res.exec_time_ns
    LAST_RESULTS = res

    out = np.empty((B, T, D), dtype=np.float32)
    for c in range(NCORES):
        out[BPC * c : BPC * (c + 1)] = res.results[c]["out"].astype(np.float32)
    return out
